# revision 1
# baseline (speedup 1.0000x reference)
"""Trainium2 Bass kernel for nn_BotRGCN2 (2-layer RGCN over 100k nodes / 600k edges).

Strategy (8 NeuronCores, SPMD):
  - Shard nodes across cores (12500/core, padded to 12544 = 98 windows of 128).
  - Feature-major (transposed) activations on-chip; node-major gather tables in
    DRAM.
  - Per RGCN layer: each core computes xw_r = x @ W_r for its own nodes
    (node-major, bf16), AllGather -> full 200704-row message table in DRAM;
    then for each owned 128-node window, dma_gather the per-edge source rows
    (transform-first messages) and scatter-add them on the PE:
    psum[feat, win] += G^T @ S with lhsT = G (gathered rows
    [128 edges x 128 feat]) and rhs = S (host-precomputed one-hot
    [128 edges x 128 window-slots] with value 1/cnt(dst,rel), streamed from
    DRAM in bf16).  Mean-per-relation is folded into the per-edge weight; the
    root term is one more accumulating matmul per window; bias via DVE.
  - Edges are preprocessed on the host: partitioned by dst owner, grouped by
    (window, src-owner-block) so every dma_gather instruction indexes a single
    <=25088-row table block (int16 index limit), padded to 128-edge chunks
    with weight-0 entries, chunk structure shared by all 8 cores so one SPMD
    program serves every core.  Gather instructions are capped at 1024 indices
    (SWDGE descriptor-carveout limit) and round-robin over 4 SWDGE queues.
  - Messages (table/G/S) run in bf16; activations/weights default to f32
    (USE_BF16 switches them to bf16 too).
"""

import sys
from contextlib import ExitStack

import numpy as np

sys.path.insert(0, "/opt/trn_rl_repo")

import ml_dtypes  # noqa: E402
import concourse.bass as bass  # noqa: E402,F401
import concourse.bacc as bacc  # noqa: E402
import concourse.mybir as mybir  # noqa: E402
import concourse.tile as tile  # noqa: E402
from concourse import library_config  # noqa: E402
from concourse.bass_utils import run_bass_kernel_spmd  # noqa: E402

C = 8           # cores
WIN = 128       # dst nodes per PSUM window
R = 2           # relations

# tunables
SG_WINDOWS = 12      # windows per gather supergroup
NIDX_CHUNKS_MAX = 8  # max 128-idx chunks per dma_gather (carveout limit)
SINGLE_PACKET = True
G_BUFS = 0           # 0 = auto (gather insts per supergroup + headroom)
S_BUFS = 2           # streamed-S supergroup buffers
PS_BUFS = 4
N_QUEUES = 4         # SWDGE queues; gathers round-robin across them
USE_BF16 = False     # bf16 activations (messages are always bf16)
TRACE = False
TMPDIR = None

F32 = mybir.dt.float32
BF16 = mybir.dt.bfloat16
LAST_RESULTS = None  # BassKernelResults of the most recent run


def _dt():
    return BF16 if USE_BF16 else F32


def _np_dt():
    return ml_dtypes.bfloat16 if USE_BF16 else np.float32


def _col_tiles(total, width):
    out = []
    c = 0
    while c < total:
        out.append((c, min(width, total - c)))
        c += width
    return out


# ----------------------------------------------------------------------------
# host-side edge preprocessing
# ----------------------------------------------------------------------------

def _edge_meta(src, dst, et, N, NPC, NP2):
    """Build the SPMD-uniform chunk structure, per-core gather index streams,
    and per-core precomputed one-hot S tiles."""
    E = src.shape[0]
    NW = NP2 // WIN
    seg = dst * R + et
    cnt = np.bincount(seg, minlength=N * R).astype(np.float64)
    w = (1.0 / np.maximum(cnt, 1.0))[seg]

    core = dst // NPC
    nl = dst % NPC
    vwin = nl // WIN
    dloc = nl % WIN
    blk = src // NPC                       # table block == src owner core
    tloc = et * NP2 + (src % NPC)          # row within block (< 2*NP2 <= 32767)
    assert 2 * NP2 <= 32768

    counts = np.zeros((C, NW, C), np.int64)
    np.add.at(counts, (core, vwin, blk), 1)
    K = -(-counts.max(axis=0) // WIN)      # [NW, C] chunks per (win, blk)

    # compute-order chunk bases (v-major, then b, then k)
    co_base = np.zeros((NW, C), np.int64)
    cc = 0
    for v in range(NW):
        for b in range(C):
            co_base[v, b] = cc
            cc += K[v, b]
    TC = int(cc)

    # gather-order (supergroup, block, window, k) + gather instruction list
    go_base = np.zeros((NW, C), np.int64)
    gather_insts = []  # (blk, start_chunk, n_chunks, sg_start)
    sg_ranges = []     # (sg_start, co_lo, co_hi)
    gc = 0
    for s0 in range(0, NW, SG_WINDOWS):
        vs = range(s0, min(s0 + SG_WINDOWS, NW))
        co_lo = int(co_base[s0, 0])
        co_hi = TC if s0 + SG_WINDOWS >= NW else int(co_base[s0 + SG_WINDOWS, 0])
        sg_ranges.append((s0, co_lo, co_hi))
        for b in range(C):
            nch = int(sum(int(K[v, b]) for v in vs))
            if nch == 0:
                continue
            off = 0
            while off < nch:
                n = min(NIDX_CHUNKS_MAX, nch - off)
                gather_insts.append((b, gc + off, n, s0))
                off += n
            for v in vs:
                go_base[v, b] = gc
                gc += K[v, b]
    assert gc == TC

    # per-core streams
    order = np.argsort((core * NW + vwin) * C + blk, kind="stable")
    gid = ((core * NW + vwin) * C + blk)[order]
    starts = np.concatenate([[0], np.cumsum(np.bincount(gid, minlength=C * NW * C))])
    rank = np.arange(E) - starts[gid]

    ce = core[order]
    v_ = vwin[order]
    b_ = blk[order]
    k_ = rank // WIN
    lane = rank % WIN

    idxg = np.zeros((C, TC * WIN), np.int16)
    cpos = (co_base[v_, b_] + k_) * WIN + lane
    gpos = (go_base[v_, b_] + k_) * WIN + lane
    idxg[ce, gpos] = tloc[order].astype(np.int16)

    # host-precomputed one-hot S (compute order): S[chunk*128+lane, dstlocal]=w
    stab = np.zeros((C, TC * WIN, WIN), ml_dtypes.bfloat16)
    stab[ce, cpos, dloc[order]] = w[order].astype(ml_dtypes.bfloat16)

    # wrap indices per gather instruction: idx i -> [i%16, off + i//16]
    TIDX = TC * WIN
    idxw = np.zeros((C, 128, TIDX // 16), np.int16)
    for (b, gc0, nch, s0) in gather_insts:
        n = nch * WIN
        segm = idxg[:, gc0 * WIN: gc0 * WIN + n].reshape(C, n // 16, 16)
        idxw[:, :16, gc0 * 8: gc0 * 8 + n // 16] = segm.transpose(0, 2, 1)
    idxw[:, 16:, :] = np.tile(idxw[:, :16, :], (1, 7, 1))

    return dict(K=K, co_base=co_base, go_base=go_base,
                gather_insts=gather_insts, sg_ranges=sg_ranges,
                TC=TC, TIDX=TIDX, NW=NW, idxw=idxw, stab=stab)


# ----------------------------------------------------------------------------
# device program
# ----------------------------------------------------------------------------

def _build_program(shapes, meta):
    DT = _dt()
    N, TW, D, OUT, NPC, NP2 = (shapes[k] for k in
                               ("N", "TW", "D", "OUT", "NPC", "NP2"))
    KT = TW // 128
    NW = meta["NW"]
    TC, TIDX = meta["TC"], meta["TIDX"]
    K, co_base, go_base = meta["K"], meta["co_base"], meta["go_base"]
    gather_insts = meta["gather_insts"]
    sg_ranges = {s0: (lo, hi) for (s0, lo, hi) in meta["sg_ranges"]}
    BR = R * NP2
    AF = mybir.ActivationFunctionType
    ALU = mybir.AluOpType

    nc = bacc.Bacc("TRN2", target_bir_lowering=False,
                   num_swdge_queues=N_QUEUES)

    twT = nc.dram_tensor("twT", [KT, 128, NP2], DT, kind="ExternalInput")
    idx16 = nc.dram_tensor("idx16", [128, TIDX // 16], mybir.dt.int16,
                           kind="ExternalInput")
    stab = nc.dram_tensor("stab", [TC, WIN, WIN], BF16, kind="ExternalInput")
    wt = nc.dram_tensor("wt", [128, KT, 128], DT, kind="ExternalInput")
    bt = nc.dram_tensor("bt", [128, 1], F32, kind="ExternalInput")
    win = nc.dram_tensor("win", [128, 128], DT, kind="ExternalInput")
    bin_ = nc.dram_tensor("bin", [128, 1], F32, kind="ExternalInput")
    wr = nc.dram_tensor("wr", [128, R * 128], DT, kind="ExternalInput")
    root = nc.dram_tensor("root", [128, 128], DT, kind="ExternalInput")
    brg = nc.dram_tensor("brg", [128, 1], F32, kind="ExternalInput")
    w1 = nc.dram_tensor("w1", [128, 128], DT, kind="ExternalInput")
    b1 = nc.dram_tensor("b1", [128, 1], F32, kind="ExternalInput")
    w2 = nc.dram_tensor("w2", [128, OUT], DT, kind="ExternalInput")
    b2 = nc.dram_tensor("b2", [OUT, 1], F32, kind="ExternalInput")
    outT = nc.dram_tensor("outT", [OUT, NP2], F32, kind="ExternalOutput")

    with tile.TileContext(nc) as tc:
        nc.gpsimd.load_library(library_config.mlp)
        with ExitStack() as stack:
            cpool = stack.enter_context(tc.tile_pool(name="const", bufs=1))
            dpool = stack.enter_context(
                tc.tile_pool(name="dram", bufs=1, space="DRAM"))
            persist = stack.enter_context(tc.tile_pool(name="persist", bufs=1))

            def cload(dram_t, shape, dtype):
                t = cpool.tile(shape, dtype, name=f"c_{dram_t.name}")
                nc.sync.dma_start(t[:], dram_t[:])
                return t

            wt_s = cload(wt, [128, KT, 128], DT)
            bt_s = cload(bt, [128, 1], F32)
            win_s = cload(win, [128, 128], DT)
            bin_s = cload(bin_, [128, 1], F32)
            wr_s = cload(wr, [128, R * 128], DT)
            root_s = cload(root, [128, 128], DT)
            brg_s = cload(brg, [128, 1], F32)
            w1_s = cload(w1, [128, 128], DT)
            b1_s = cload(b1, [128, 1], F32)
            w2_s = cload(w2, [128, OUT], DT)
            b2_s = cload(b2, [OUT, 1], F32)
            idx_s = cload(idx16, [128, TIDX // 16], mybir.dt.int16)

            tables = [dpool.tile([C * BR, 128], BF16, addr_space="Shared",
                                 name=f"table{i}") for i in range(2)]
            agin = dpool.tile([R, NP2, 128], BF16, name="agin")

            xT = persist.tile([128, NP2], DT, name="xT")

            # ---------------- stage 1: x = lrelu(lrelu(tweet@Wt+bt)@Win+bin)
            with tc.tile_pool(name="s1", bufs=2) as s1p, \
                 tc.tile_pool(name="ps1", bufs=2, space="PSUM") as ps1:
                for (c0, fw) in _col_tiles(NP2, 512):
                    twt = s1p.tile([128, KT, fw], DT, tag="twt", name="twt")
                    nc.sync.dma_start(
                        twt[:], twT[:, :, c0:c0 + fw].rearrange("k p f -> p k f"))
                    ps_t = ps1.tile([128, fw], F32, tag="pst", name="ps_t")
                    for k in range(KT):
                        nc.tensor.matmul(ps_t[:], wt_s[:, k, :], twt[:, k, :],
                                         start=(k == 0), stop=(k == KT - 1))
                    tt = s1p.tile([128, fw], DT, tag="tt", name="tt")
                    nc.scalar.activation(tt[:], ps_t[:], AF.Lrelu,
                                         bias=bt_s[:], alpha=0.01)
                    ps_x = ps1.tile([128, fw], F32, tag="psx", name="ps_x")
                    nc.tensor.matmul(ps_x[:], win_s[:], tt[:],
                                     start=True, stop=True)
                    nc.scalar.activation(xT[:, c0:c0 + fw], ps_x[:], AF.Lrelu,
                                         bias=bin_s[:], alpha=0.01)

            # ---------------- 2 RGCN layers
            for layer in range(2):
                table = tables[layer]
                # phase A: local xw table shard (bf16) + AllGather
                with tc.tile_pool(name=f"pa{layer}", bufs=3) as pap, \
                     tc.tile_pool(name=f"psa{layer}", bufs=2,
                                  space="PSUM") as psa:
                    for nt in range(NW):
                        psA = psa.tile([128, R * 128], F32, tag="psA",
                                       name="psA")
                        nc.tensor.matmul(psA[:], xT[:, nt * 128:(nt + 1) * 128],
                                         wr_s[:], start=True, stop=True)
                        ob = pap.tile([128, R, 128], BF16, tag="ob", name="ob")
                        nc.scalar.activation(
                            ob[:].rearrange("p e f -> p (e f)"), psA[:],
                            AF.Copy)
                        nc.sync.dma_start(
                            agin[:, nt * 128:(nt + 1) * 128, :]
                            .rearrange("e n f -> n e f"), ob[:])
                    nc.gpsimd.collective_compute(
                        "AllGather", mybir.AluOpType.bypass,
                        replica_groups=[list(range(C))],
                        ins=[agin[:].rearrange("e n f -> (e n) f")],
                        outs=[table[:]])

                # phase B: gather + one-hot scatter matmuls per window
                per_sg = {}
                for (b, gc0, nch, s0) in gather_insts:
                    per_sg[s0] = per_sg.get(s0, 0) + 1
                g_bufs = G_BUFS or (max(per_sg.values()) + 4)
                with tc.tile_pool(name=f"g{layer}", bufs=g_bufs) as gp, \
                     tc.tile_pool(name=f"s{layer}", bufs=S_BUFS) as sp, \
                     tc.tile_pool(name=f"pb{layer}", bufs=PS_BUFS,
                                  space="PSUM") as pb:
                    by_sg = {}
                    for gi, (b, gc0, nch, s0) in enumerate(gather_insts):
                        by_sg.setdefault(s0, []).append((b, gc0, nch, gi))
                    for s0 in range(0, NW, SG_WINDOWS):
                        vs = range(s0, min(s0 + SG_WINDOWS, NW))
                        co_lo, co_hi = sg_ranges[s0]
                        ssg = sp.tile([128, co_hi - co_lo, WIN], BF16,
                                      tag="ssg", name="ssg")
                        nc.sync.dma_start(
                            ssg[:],
                            stab[co_lo:co_hi].rearrange("c e n -> e c n"))
                        gts = {}
                        for (b, gc0, nch, gi) in by_sg.get(s0, []):
                            gt = gp.tile([128, nch, 128], BF16, tag="g",
                                         name="gt")
                            nc.gpsimd.dma_gather(
                                gt[:], table[b * BR:(b + 1) * BR, :],
                                idx_s[:, gc0 * 8: (gc0 + nch) * 8],
                                nch * WIN, nch * WIN, 128,
                                single_packet=SINGLE_PACKET,
                                queue_num=gi % N_QUEUES)
                            gts.setdefault(b, []).append((gt, gc0, nch))
                        for v in vs:
                            ps = pb.tile([128, WIN], F32, tag="psb", name="psb")
                            i = 0
                            for b in range(C):
                                for k in range(int(K[v, b])):
                                    ccx = int(co_base[v, b]) + k
                                    cg = int(go_base[v, b]) + k
                                    gt = None
                                    for (g_t, g_0, g_n) in gts[b]:
                                        if g_0 <= cg < g_0 + g_n:
                                            gt, j = g_t, cg - g_0
                                            break
                                    nc.tensor.matmul(
                                        ps[:], gt[:, j, :],
                                        ssg[:, ccx - co_lo, :],
                                        start=(i == 0), stop=False)
                                    i += 1
                            nc.tensor.matmul(ps[:], root_s[:],
                                             xT[:, v * 128:(v + 1) * 128],
                                             start=(i == 0), stop=True,
                                             skip_group_check=True)
                            nc.vector.tensor_scalar(
                                xT[:, v * 128:(v + 1) * 128], ps[:],
                                brg_s[:], None, op0=ALU.add)

            # ---------------- head
            with tc.tile_pool(name="hd", bufs=3) as hp, \
                 tc.tile_pool(name="psh", bufs=2, space="PSUM") as psh, \
                 tc.tile_pool(name="outp", bufs=1) as outp:
                outT_s = outp.tile([OUT, NP2], F32, name="outT_s")
                for (c0, fw) in _col_tiles(NP2, 512):
                    ph = psh.tile([128, fw], F32, tag="ph", name="ph")
                    nc.tensor.matmul(ph[:], w1_s[:], xT[:, c0:c0 + fw],
                                     start=True, stop=True)
                    ht = hp.tile([128, fw], DT, tag="ht", name="ht")
                    nc.scalar.activation(ht[:], ph[:], AF.Lrelu,
                                         bias=b1_s[:], alpha=0.01)
                    po = psh.tile([OUT, fw], F32, tag="po", name="po")
                    nc.tensor.matmul(po[:], w2_s[:], ht[:],
                                     start=True, stop=True)
                    nc.vector.tensor_scalar(outT_s[:, c0:c0 + fw], po[:],
                                            b2_s[:], None, op0=ALU.add)
                nc.sync.dma_start(outT[:, :], outT_s[:])

    nc.compile()
    return nc


# ----------------------------------------------------------------------------
# entry point
# ----------------------------------------------------------------------------

def kernel(**inputs):
    global LAST_RESULTS
    tweet = np.asarray(inputs["tweet"], np.float32)
    ei = np.asarray(inputs["edge_index"]).astype(np.int64)
    et = np.asarray(inputs["edge_type"]).astype(np.int64)
    W_tweet = np.asarray(inputs["W_tweet"], np.float32)
    b_tweet = np.asarray(inputs["b_tweet"], np.float32)
    W_in = np.asarray(inputs["W_in"], np.float32)
    b_in = np.asarray(inputs["b_in"], np.float32)
    rgcn_weight = np.asarray(inputs["rgcn_weight"], np.float32)
    rgcn_root = np.asarray(inputs["rgcn_root"], np.float32)
    rgcn_bias = np.asarray(inputs["rgcn_bias"], np.float32)
    W_out1 = np.asarray(inputs["W_out1"], np.float32)
    b_out1 = np.asarray(inputs["b_out1"], np.float32)
    W_out2 = np.asarray(inputs["W_out2"], np.float32)
    b_out2 = np.asarray(inputs["b_out2"], np.float32)

    N, TW = tweet.shape
    D = W_in.shape[0]
    OUT = W_out2.shape[1]
    assert N % C == 0 and TW % 128 == 0 and D == 128
    NPC = N // C
    NP2 = -(-NPC // WIN) * WIN
    src, dst = ei[0], ei[1]

    meta = _edge_meta(src, dst, et, N, NPC, NP2)
    shapes = dict(N=N, TW=TW, D=D, OUT=OUT, NPC=NPC, NP2=NP2)
    npdt = _np_dt()
    KT = TW // 128

    nc = _build_program(shapes, meta)

    shared = {
        "wt": np.ascontiguousarray(
            W_tweet.reshape(KT, 128, 128).transpose(1, 0, 2)).astype(npdt),
        "bt": b_tweet.reshape(128, 1),
        "win": W_in.astype(npdt),
        "bin": b_in.reshape(128, 1),
        "wr": np.ascontiguousarray(
            rgcn_weight.transpose(1, 0, 2).reshape(128, R * 128)).astype(npdt),
        "root": rgcn_root.astype(npdt),
        "brg": rgcn_bias.reshape(128, 1),
        "w1": W_out1.astype(npdt),
        "b1": b_out1.reshape(128, 1),
        "w2": W_out2.astype(npdt),
        "b2": b_out2.reshape(OUT, 1),
    }

    in_maps = []
    for c in range(C):
        tw_c = np.zeros((KT, 128, NP2), npdt)
        tw_c[:, :, :NPC] = (tweet[c * NPC:(c + 1) * NPC].T
                            .reshape(KT, 128, NPC).astype(npdt))
        m = dict(shared)
        m["twT"] = tw_c
        m["idx16"] = meta["idxw"][c]
        m["stab"] = meta["stab"][c].reshape(meta["TC"], WIN, WIN)
        in_maps.append(m)

    res = run_bass_kernel_spmd(nc, in_maps, core_ids=list(range(C)),
                               trace=TRACE, tmpdir=TMPDIR)
    LAST_RESULTS = res

    out = np.zeros((N, OUT), np.float32)
    for c in range(C):
        out[c * NPC:(c + 1) * NPC] = res.results[c]["outT"][:, :NPC].T
    return out



# revision 10
# speedup vs baseline: 1.1886x; 1.1886x over previous
"""Trainium2 Bass kernel for nn_BotRGCN2 (2-layer RGCN over 100k nodes / 600k edges).

Strategy (8 NeuronCores, SPMD):
  - Shard nodes across cores (12500/core, padded to 12544 = 98 windows of 128).
  - Feature-major (transposed) activations on-chip; node-major gather tables in
    DRAM.
  - Per RGCN layer: each core computes xw_r = x @ W_r for its own nodes
    (node-major, bf16), AllGather -> full 200704-row message table in DRAM;
    then for each owned 128-node window, dma_gather the per-edge source rows
    (transform-first messages) and scatter-add them on the PE:
    psum[feat, win] += G^T @ S with lhsT = G (gathered rows
    [128 edges x 128 feat]) and rhs = S (host-precomputed one-hot
    [128 edges x 128 window-slots] with value 1/cnt(dst,rel), streamed from
    DRAM in bf16).  Mean-per-relation is folded into the per-edge weight; the
    root term is one more accumulating matmul per window; bias via DVE.
  - Edges are preprocessed on the host: partitioned by dst owner, grouped by
    (window, src-owner-block) so every dma_gather instruction indexes a single
    <=25088-row table block (int16 index limit), padded to 128-edge chunks
    with weight-0 entries, chunk structure shared by all 8 cores so one SPMD
    program serves every core.  Gather instructions are capped at 1024 indices
    (SWDGE descriptor-carveout limit) and round-robin over 4 SWDGE queues.
  - Messages (table/G/S) run in bf16; activations/weights default to f32
    (USE_BF16 switches them to bf16 too).
"""

import sys
from contextlib import ExitStack

import numpy as np

sys.path.insert(0, "/opt/trn_rl_repo")

import ml_dtypes  # noqa: E402
import concourse.bass as bass  # noqa: E402,F401
import concourse.bacc as bacc  # noqa: E402
import concourse.mybir as mybir  # noqa: E402
import concourse.tile as tile  # noqa: E402
from concourse import library_config  # noqa: E402
from concourse.bass_utils import run_bass_kernel_spmd  # noqa: E402

C = 8           # cores
WIN = 128       # dst nodes per PSUM window
R = 2           # relations

# tunables
SG_WINDOWS = 12      # windows per gather supergroup
NIDX_CHUNKS_MAX = 8  # max 128-idx chunks per dma_gather (carveout limit)
SINGLE_PACKET = True
G_BUFS = 0           # 0 = auto (gather insts per supergroup + headroom)
S_BUFS = 2           # streamed-S supergroup buffers
PS_BUFS = 4
N_QUEUES = 4         # SWDGE queues; gathers round-robin across them
USE_BF16 = True      # bf16 activations (messages are always bf16)
TRACE = False
TMPDIR = None

F32 = mybir.dt.float32
BF16 = mybir.dt.bfloat16
LAST_RESULTS = None  # BassKernelResults of the most recent run


def _dt():
    return BF16 if USE_BF16 else F32


def _np_dt():
    return ml_dtypes.bfloat16 if USE_BF16 else np.float32


def _col_tiles(total, width):
    out = []
    c = 0
    while c < total:
        out.append((c, min(width, total - c)))
        c += width
    return out


# ----------------------------------------------------------------------------
# host-side edge preprocessing
# ----------------------------------------------------------------------------

def _edge_meta(src, dst, et, N, NPC, NP2):
    """Build the SPMD-uniform chunk structure, per-core gather index streams,
    and per-core precomputed one-hot S tiles."""
    E = src.shape[0]
    NW = NP2 // WIN
    seg = dst * R + et
    cnt = np.bincount(seg, minlength=N * R).astype(np.float64)
    w = (1.0 / np.maximum(cnt, 1.0))[seg]

    core = dst // NPC
    nl = dst % NPC
    vwin = nl // WIN
    dloc = nl % WIN
    blk = src // NPC                       # table block == src owner core
    tloc = (src % NPC) * R + et            # row within block (< 2*NP2 <= 32767)
    assert 2 * NP2 <= 32768

    counts = np.zeros((C, NW, C), np.int64)
    np.add.at(counts, (core, vwin, blk), 1)
    K = -(-counts.max(axis=0) // WIN)      # [NW, C] chunks per (win, blk)

    # compute-order chunk bases (v-major, then b, then k)
    co_base = np.zeros((NW, C), np.int64)
    cc = 0
    for v in range(NW):
        for b in range(C):
            co_base[v, b] = cc
            cc += K[v, b]
    TC = int(cc)

    # gather-order (supergroup, block, window, k) + gather instruction list
    go_base = np.zeros((NW, C), np.int64)
    gather_insts = []  # (blk, start_chunk, n_chunks, sg_start)
    sg_ranges = []     # (sg_start, co_lo, co_hi)
    gc = 0
    for s0 in range(0, NW, SG_WINDOWS):
        vs = range(s0, min(s0 + SG_WINDOWS, NW))
        co_lo = int(co_base[s0, 0])
        co_hi = TC if s0 + SG_WINDOWS >= NW else int(co_base[s0 + SG_WINDOWS, 0])
        sg_ranges.append((s0, co_lo, co_hi))
        for b in range(C):
            nch = int(sum(int(K[v, b]) for v in vs))
            if nch == 0:
                continue
            off = 0
            while off < nch:
                n = min(NIDX_CHUNKS_MAX, nch - off)
                gather_insts.append((b, gc + off, n, s0))
                off += n
            for v in vs:
                go_base[v, b] = gc
                gc += K[v, b]
    assert gc == TC

    # per-core streams
    order = np.argsort((core * NW + vwin) * C + blk, kind="stable")
    gid = ((core * NW + vwin) * C + blk)[order]
    starts = np.concatenate([[0], np.cumsum(np.bincount(gid, minlength=C * NW * C))])
    rank = np.arange(E) - starts[gid]

    ce = core[order]
    v_ = vwin[order]
    b_ = blk[order]
    k_ = rank // WIN
    lane = rank % WIN

    idxg = np.zeros((C, TC * WIN), np.int16)
    cpos = (co_base[v_, b_] + k_) * WIN + lane
    gpos = (go_base[v_, b_] + k_) * WIN + lane
    idxg[ce, gpos] = tloc[order].astype(np.int16)

    # host-precomputed one-hot S, lane-major for contiguous device DMA:
    # S[lane, chunk, dstlocal] = w
    stab = np.zeros((C, WIN, TC, WIN), ml_dtypes.bfloat16)
    stab[ce, lane, co_base[v_, b_] + k_, dloc[order]] = \
        w[order].astype(ml_dtypes.bfloat16)

    # wrap indices per gather instruction: idx i -> [i%16, off + i//16]
    TIDX = TC * WIN
    idxw = np.zeros((C, 128, TIDX // 16), np.int16)
    for (b, gc0, nch, s0) in gather_insts:
        n = nch * WIN
        segm = idxg[:, gc0 * WIN: gc0 * WIN + n].reshape(C, n // 16, 16)
        idxw[:, :16, gc0 * 8: gc0 * 8 + n // 16] = segm.transpose(0, 2, 1)
    idxw[:, 16:, :] = np.tile(idxw[:, :16, :], (1, 7, 1))

    return dict(K=K, co_base=co_base, go_base=go_base,
                gather_insts=gather_insts, sg_ranges=sg_ranges,
                TC=TC, TIDX=TIDX, NW=NW, idxw=idxw, stab=stab)


# ----------------------------------------------------------------------------
# device program
# ----------------------------------------------------------------------------

def _build_program(shapes, meta):
    DT = _dt()
    N, TW, D, OUT, NPC, NP2 = (shapes[k] for k in
                               ("N", "TW", "D", "OUT", "NPC", "NP2"))
    KT = TW // 128
    NW = meta["NW"]
    TC, TIDX = meta["TC"], meta["TIDX"]
    K, co_base, go_base = meta["K"], meta["co_base"], meta["go_base"]
    gather_insts = meta["gather_insts"]
    sg_ranges = {s0: (lo, hi) for (s0, lo, hi) in meta["sg_ranges"]}
    BR = R * NP2
    AF = mybir.ActivationFunctionType
    ALU = mybir.AluOpType

    nc = bacc.Bacc("TRN2", target_bir_lowering=False,
                   num_swdge_queues=N_QUEUES)

    twT = nc.dram_tensor("twT", [KT, 128, NP2], DT, kind="ExternalInput")
    idx16 = nc.dram_tensor("idx16", [128, TIDX // 16], mybir.dt.int16,
                           kind="ExternalInput")
    stab = nc.dram_tensor("stab", [WIN, TC, WIN], BF16, kind="ExternalInput")
    wt = nc.dram_tensor("wt", [128, KT, 128], DT, kind="ExternalInput")
    bt = nc.dram_tensor("bt", [128, 1], F32, kind="ExternalInput")
    win = nc.dram_tensor("win", [128, 128], DT, kind="ExternalInput")
    bin_ = nc.dram_tensor("bin", [128, 1], F32, kind="ExternalInput")
    wr = nc.dram_tensor("wr", [128, R * 128], DT, kind="ExternalInput")
    root = nc.dram_tensor("root", [128, 128], DT, kind="ExternalInput")
    brg = nc.dram_tensor("brg", [128, 1], F32, kind="ExternalInput")
    w1 = nc.dram_tensor("w1", [128, 128], DT, kind="ExternalInput")
    b1 = nc.dram_tensor("b1", [128, 1], F32, kind="ExternalInput")
    w2 = nc.dram_tensor("w2", [128, OUT], DT, kind="ExternalInput")
    b2 = nc.dram_tensor("b2", [OUT, 1], F32, kind="ExternalInput")
    outT = nc.dram_tensor("outT", [OUT, NP2], F32, kind="ExternalOutput")

    with tile.TileContext(nc) as tc:
        nc.gpsimd.load_library(library_config.mlp)
        with ExitStack() as stack:
            cpool = stack.enter_context(tc.tile_pool(name="const", bufs=1))
            dpool = stack.enter_context(
                tc.tile_pool(name="dram", bufs=1, space="DRAM"))
            persist = stack.enter_context(tc.tile_pool(name="persist", bufs=1))

            def cload(dram_t, shape, dtype):
                t = cpool.tile(shape, dtype, name=f"c_{dram_t.name}")
                nc.sync.dma_start(t[:], dram_t[:])
                return t

            wt_s = cload(wt, [128, KT, 128], DT)
            bt_s = cload(bt, [128, 1], F32)
            win_s = cload(win, [128, 128], DT)
            bin_s = cload(bin_, [128, 1], F32)
            wr_s = cload(wr, [128, R * 128], DT)
            root_s = cload(root, [128, 128], DT)
            brg_s = cload(brg, [128, 1], F32)
            w1_s = cload(w1, [128, 128], DT)
            b1_s = cload(b1, [128, 1], F32)
            w2_s = cload(w2, [128, OUT], DT)
            b2_s = cload(b2, [OUT, 1], F32)
            idx_s = cload(idx16, [128, TIDX // 16], mybir.dt.int16)

            tables = [dpool.tile([C * BR, 128], BF16, addr_space="Shared",
                                 name=f"table{i}") for i in range(2)]
            agin = dpool.tile([NP2, R, 128], BF16, name="agin")

            xT = persist.tile([128, NP2], DT, name="xT")

            # ---------------- stage 1: x = lrelu(lrelu(tweet@Wt+bt)@Win+bin)
            with tc.tile_pool(name="s1", bufs=2) as s1p, \
                 tc.tile_pool(name="ps1", bufs=2, space="PSUM") as ps1:
                for (c0, fw) in _col_tiles(NP2, 512):
                    twt = s1p.tile([128, KT, fw], DT, tag="twt", name="twt")
                    nc.sync.dma_start(
                        twt[:], twT[:, :, c0:c0 + fw].rearrange("k p f -> p k f"))
                    ps_t = ps1.tile([128, fw], F32, tag="pst", name="ps_t")
                    for k in range(KT):
                        nc.tensor.matmul(ps_t[:], wt_s[:, k, :], twt[:, k, :],
                                         start=(k == 0), stop=(k == KT - 1))
                    tt = s1p.tile([128, fw], DT, tag="tt", name="tt")
                    nc.scalar.activation(tt[:], ps_t[:], AF.Lrelu,
                                         bias=bt_s[:], alpha=0.01)
                    ps_x = ps1.tile([128, fw], F32, tag="psx", name="ps_x")
                    nc.tensor.matmul(ps_x[:], win_s[:], tt[:],
                                     start=True, stop=True)
                    nc.scalar.activation(xT[:, c0:c0 + fw], ps_x[:], AF.Lrelu,
                                         bias=bin_s[:], alpha=0.01)

            # ---------------- 2 RGCN layers
            for layer in range(2):
                table = tables[layer]
                # phase A: local xw table shard (bf16) + AllGather
                with tc.tile_pool(name=f"pa{layer}", bufs=3) as pap, \
                     tc.tile_pool(name=f"psa{layer}", bufs=2,
                                  space="PSUM") as psa:
                    for nt in range(NW):
                        psA = psa.tile([128, R * 128], F32, tag="psA",
                                       name="psA")
                        nc.tensor.matmul(psA[:], xT[:, nt * 128:(nt + 1) * 128],
                                         wr_s[:], start=True, stop=True)
                        ob = pap.tile([128, R, 128], BF16, tag="ob", name="ob")
                        nc.scalar.activation(
                            ob[:].rearrange("p e f -> p (e f)"), psA[:],
                            AF.Copy)
                        nc.sync.dma_start(
                            agin[nt * 128:(nt + 1) * 128, :, :], ob[:])
                    nc.gpsimd.collective_compute(
                        "AllGather", mybir.AluOpType.bypass,
                        replica_groups=[list(range(C))],
                        ins=[agin[:].rearrange("n e f -> (n e) f")],
                        outs=[table[:]])

                # phase B: gather + one-hot scatter matmuls per window
                per_sg = {}
                for (b, gc0, nch, s0) in gather_insts:
                    per_sg[s0] = per_sg.get(s0, 0) + 1
                g_bufs = G_BUFS or (max(per_sg.values()) + 4)
                with tc.tile_pool(name=f"g{layer}", bufs=g_bufs) as gp, \
                     tc.tile_pool(name=f"s{layer}", bufs=S_BUFS) as sp, \
                     tc.tile_pool(name=f"pb{layer}", bufs=PS_BUFS,
                                  space="PSUM") as pb:
                    by_sg = {}
                    for gi, (b, gc0, nch, s0) in enumerate(gather_insts):
                        by_sg.setdefault(s0, []).append((b, gc0, nch, gi))
                    for s0 in range(0, NW, SG_WINDOWS):
                        vs = range(s0, min(s0 + SG_WINDOWS, NW))
                        co_lo, co_hi = sg_ranges[s0]
                        ssg = sp.tile([128, co_hi - co_lo, WIN], BF16,
                                      tag="ssg", name="ssg")
                        nc.sync.dma_start(ssg[:], stab[:, co_lo:co_hi, :])
                        gts = {}
                        for (b, gc0, nch, gi) in by_sg.get(s0, []):
                            gt = gp.tile([128, nch, 128], BF16, tag="g",
                                         name="gt")
                            nc.gpsimd.dma_gather(
                                gt[:], table[b * BR:(b + 1) * BR, :],
                                idx_s[:, gc0 * 8: (gc0 + nch) * 8],
                                nch * WIN, nch * WIN, 128,
                                single_packet=SINGLE_PACKET,
                                queue_num=gi % N_QUEUES)
                            gts.setdefault(b, []).append((gt, gc0, nch))
                        for v in vs:
                            ps = pb.tile([128, WIN], F32, tag="psb", name="psb")
                            i = 0
                            for b in range(C):
                                for k in range(int(K[v, b])):
                                    ccx = int(co_base[v, b]) + k
                                    cg = int(go_base[v, b]) + k
                                    gt = None
                                    for (g_t, g_0, g_n) in gts[b]:
                                        if g_0 <= cg < g_0 + g_n:
                                            gt, j = g_t, cg - g_0
                                            break
                                    nc.tensor.matmul(
                                        ps[:], gt[:, j, :],
                                        ssg[:, ccx - co_lo, :],
                                        start=(i == 0), stop=False)
                                    i += 1
                            nc.tensor.matmul(ps[:], root_s[:],
                                             xT[:, v * 128:(v + 1) * 128],
                                             start=(i == 0), stop=True,
                                             skip_group_check=True)
                            nc.vector.tensor_scalar(
                                xT[:, v * 128:(v + 1) * 128], ps[:],
                                brg_s[:], None, op0=ALU.add)

            # ---------------- head
            with tc.tile_pool(name="hd", bufs=3) as hp, \
                 tc.tile_pool(name="psh", bufs=2, space="PSUM") as psh, \
                 tc.tile_pool(name="outp", bufs=1) as outp:
                outT_s = outp.tile([OUT, NP2], F32, name="outT_s")
                for (c0, fw) in _col_tiles(NP2, 512):
                    ph = psh.tile([128, fw], F32, tag="ph", name="ph")
                    nc.tensor.matmul(ph[:], w1_s[:], xT[:, c0:c0 + fw],
                                     start=True, stop=True)
                    ht = hp.tile([128, fw], DT, tag="ht", name="ht")
                    nc.scalar.activation(ht[:], ph[:], AF.Lrelu,
                                         bias=b1_s[:], alpha=0.01)
                    po = psh.tile([OUT, fw], F32, tag="po", name="po")
                    nc.tensor.matmul(po[:], w2_s[:], ht[:],
                                     start=True, stop=True)
                    nc.vector.tensor_scalar(outT_s[:, c0:c0 + fw], po[:],
                                            b2_s[:], None, op0=ALU.add)
                nc.sync.dma_start(outT[:, :], outT_s[:])

    nc.compile()
    return nc


# ----------------------------------------------------------------------------
# entry point
# ----------------------------------------------------------------------------

def kernel(**inputs):
    global LAST_RESULTS
    tweet = np.asarray(inputs["tweet"], np.float32)
    ei = np.asarray(inputs["edge_index"]).astype(np.int64)
    et = np.asarray(inputs["edge_type"]).astype(np.int64)
    W_tweet = np.asarray(inputs["W_tweet"], np.float32)
    b_tweet = np.asarray(inputs["b_tweet"], np.float32)
    W_in = np.asarray(inputs["W_in"], np.float32)
    b_in = np.asarray(inputs["b_in"], np.float32)
    rgcn_weight = np.asarray(inputs["rgcn_weight"], np.float32)
    rgcn_root = np.asarray(inputs["rgcn_root"], np.float32)
    rgcn_bias = np.asarray(inputs["rgcn_bias"], np.float32)
    W_out1 = np.asarray(inputs["W_out1"], np.float32)
    b_out1 = np.asarray(inputs["b_out1"], np.float32)
    W_out2 = np.asarray(inputs["W_out2"], np.float32)
    b_out2 = np.asarray(inputs["b_out2"], np.float32)

    N, TW = tweet.shape
    D = W_in.shape[0]
    OUT = W_out2.shape[1]
    assert N % C == 0 and TW % 128 == 0 and D == 128
    NPC = N // C
    NP2 = -(-NPC // WIN) * WIN
    src, dst = ei[0], ei[1]

    meta = _edge_meta(src, dst, et, N, NPC, NP2)
    shapes = dict(N=N, TW=TW, D=D, OUT=OUT, NPC=NPC, NP2=NP2)
    npdt = _np_dt()
    KT = TW // 128

    nc = _build_program(shapes, meta)

    shared = {
        "wt": np.ascontiguousarray(
            W_tweet.reshape(KT, 128, 128).transpose(1, 0, 2)).astype(npdt),
        "bt": b_tweet.reshape(128, 1),
        "win": W_in.astype(npdt),
        "bin": b_in.reshape(128, 1),
        "wr": np.ascontiguousarray(
            rgcn_weight.transpose(1, 0, 2).reshape(128, R * 128)).astype(npdt),
        "root": rgcn_root.astype(npdt),
        "brg": rgcn_bias.reshape(128, 1),
        "w1": W_out1.astype(npdt),
        "b1": b_out1.reshape(128, 1),
        "w2": W_out2.astype(npdt),
        "b2": b_out2.reshape(OUT, 1),
    }

    in_maps = []
    for c in range(C):
        tw_c = np.zeros((KT, 128, NP2), npdt)
        tw_c[:, :, :NPC] = (tweet[c * NPC:(c + 1) * NPC].T
                            .reshape(KT, 128, NPC).astype(npdt))
        m = dict(shared)
        m["twT"] = tw_c
        m["idx16"] = meta["idxw"][c]
        m["stab"] = meta["stab"][c]
        in_maps.append(m)

    res = run_bass_kernel_spmd(nc, in_maps, core_ids=list(range(C)),
                               trace=TRACE, tmpdir=TMPDIR)
    LAST_RESULTS = res

    out = np.zeros((N, OUT), np.float32)
    for c in range(C):
        out[c * NPC:(c + 1) * NPC] = res.results[c]["outT"][:, :NPC].T
    return out



# revision 14
# speedup vs baseline: 1.3278x; 1.1172x over previous
"""Trainium2 Bass kernel for nn_BotRGCN2 (2-layer RGCN over 100k nodes / 600k edges).

Strategy (8 NeuronCores, SPMD):
  - Shard nodes across cores (12500/core, padded to 12544 = 98 windows of 128).
  - Feature-major (transposed) activations on-chip; node-major RAW-x gather
    table in DRAM (one 256B bf16 row per node -> AllGather output is
    C*NP2 x 128 = 25.7MB, half the transformed-table variant).
  - Per RGCN layer: each core transposes its xT windows (PE transpose) to
    node-major bf16 rows -> agin -> AllGather -> full raw-x table in DRAM.
    For each 128-dst-node window, dma_gather the per-edge source rows and
    scatter-add on the PE into a relation-split psum [128 feat, 256]:
      psum[:, r*128+dst] += G_w^T @ S'  with lhsT = G_w (gathered rows
    scaled in-place by w=1/cnt via a broadcast tensor_tensor) and
    rhs = S' (built ON DEVICE per 4-window group: one is_equal
    tensor_tensor of a broadcast iota [0..256) against per-lane dstcol =
    dloc + 128*rel; pad lanes have dstcol=-1 -> all-zero rows).
    Then per window: copy psum -> aggS bf16 and apply the per-relation
    weights + root with 3 accumulating matmuls:
      out = W_0^T aggS_0 + W_1^T aggS_1 + root^T xT_win (+ bias via DVE).
  - Layer-(l+1) phase A (transpose+copy+DMA of each finished window) is
    interleaved into stage 1 / layer-l phase B so the AllGather fires
    immediately after the producing phase ends.
  - Edges are preprocessed on the host: partitioned by dst owner, grouped by
    (window, src-owner-block) into 128-lane chunks, gather instructions per
    (supergroup, block) capped at 1024 indices, round-robin over 4 SWDGE
    queues.  S-structure ships as per-lane (dstcol, w) streams (tiny)
    instead of dense one-hot tables.
"""

import sys
from contextlib import ExitStack

import numpy as np

sys.path.insert(0, "/opt/trn_rl_repo")

import ml_dtypes  # noqa: E402
import concourse.bass as bass  # noqa: E402,F401
import concourse.bacc as bacc  # noqa: E402
import concourse.mybir as mybir  # noqa: E402
import concourse.tile as tile  # noqa: E402
from concourse import library_config  # noqa: E402
from concourse.bass_utils import run_bass_kernel_spmd  # noqa: E402

C = 8           # cores
WIN = 128       # dst nodes per window
R = 2           # relations

# tunables
SG_WINDOWS = 12      # windows per gather supergroup
BG_WINDOWS = 4       # windows per on-device S'-build group (divides SG_WINDOWS)
NIDX_CHUNKS_MAX = 8  # max 128-idx chunks per dma_gather (carveout limit)
SINGLE_PACKET = True
G_BUFS = 0           # 0 = auto (gather insts per supergroup + headroom)
S_BUFS = 3           # S' build-group buffers
PS_BUFS = 4
N_QUEUES = 4         # SWDGE queues; gathers round-robin across them
USE_BF16 = True      # bf16 activations (messages are always bf16)
TRACE = False
TMPDIR = None

F32 = mybir.dt.float32
BF16 = mybir.dt.bfloat16
LAST_RESULTS = None  # BassKernelResults of the most recent run


def _dt():
    return BF16 if USE_BF16 else F32


def _np_dt():
    return ml_dtypes.bfloat16 if USE_BF16 else np.float32


def _col_tiles(total, width):
    out = []
    c = 0
    while c < total:
        out.append((c, min(width, total - c)))
        c += width
    return out


# ----------------------------------------------------------------------------
# host-side edge preprocessing
# ----------------------------------------------------------------------------

def _edge_meta(src, dst, et, N, NPC, NP2):
    """Build the SPMD-uniform chunk structure, per-core gather index streams,
    and per-core per-lane (dstcol, w) streams for on-device S' builds."""
    E = src.shape[0]
    NW = NP2 // WIN
    seg = dst * R + et
    cnt = np.bincount(seg, minlength=N * R).astype(np.float64)
    w = (1.0 / np.maximum(cnt, 1.0))[seg]

    core = dst // NPC
    nl = dst % NPC
    vwin = nl // WIN
    dloc = nl % WIN
    blk = src // NPC                       # table block == src owner core
    tloc = src % NPC                       # row within raw-x block (< NP2)
    assert NP2 <= 32768

    counts = np.zeros((C, NW, C), np.int64)
    np.add.at(counts, (core, vwin, blk), 1)
    K = -(-counts.max(axis=0) // WIN)      # [NW, C] chunks per (win, blk)

    # compute-order chunk bases (v-major, then b, then k)
    co_base = np.zeros((NW, C), np.int64)
    cc = 0
    for v in range(NW):
        for b in range(C):
            co_base[v, b] = cc
            cc += K[v, b]
    TC = int(cc)

    # gather-order (supergroup, block, window, k) + gather instruction list
    go_base = np.zeros((NW, C), np.int64)
    gather_insts = []  # (blk, start_chunk, n_chunks, sg_start)
    sg_ranges = []     # (sg_start, co_lo, co_hi)
    gc = 0
    for s0 in range(0, NW, SG_WINDOWS):
        vs = range(s0, min(s0 + SG_WINDOWS, NW))
        co_lo = int(co_base[s0, 0])
        co_hi = TC if s0 + SG_WINDOWS >= NW else int(co_base[s0 + SG_WINDOWS, 0])
        sg_ranges.append((s0, co_lo, co_hi))
        for b in range(C):
            nch = int(sum(int(K[v, b]) for v in vs))
            if nch == 0:
                continue
            off = 0
            while off < nch:
                n = min(NIDX_CHUNKS_MAX, nch - off)
                gather_insts.append((b, gc + off, n, s0))
                off += n
            for v in vs:
                go_base[v, b] = gc
                gc += K[v, b]
    assert gc == TC

    # build-group (BG_WINDOWS) compute-order chunk ranges
    bg_ranges = []     # (bg_start, co_lo, co_hi)
    for b0 in range(0, NW, BG_WINDOWS):
        lo = int(co_base[b0, 0])
        hi = TC if b0 + BG_WINDOWS >= NW else int(co_base[b0 + BG_WINDOWS, 0])
        bg_ranges.append((b0, lo, hi))

    # per-core streams
    order = np.argsort((core * NW + vwin) * C + blk, kind="stable")
    gid = ((core * NW + vwin) * C + blk)[order]
    starts = np.concatenate([[0], np.cumsum(np.bincount(gid, minlength=C * NW * C))])
    rank = np.arange(E) - starts[gid]

    ce = core[order]
    v_ = vwin[order]
    b_ = blk[order]
    k_ = rank // WIN
    lane = rank % WIN

    idxg = np.zeros((C, TC * WIN), np.int16)
    gpos = (go_base[v_, b_] + k_) * WIN + lane
    idxg[ce, gpos] = tloc[order].astype(np.int16)

    # per-lane S' structure: dstcol (compute order; -1 on pad lanes) and
    # w (gather order; 0 on pad lanes)
    dstc = np.full((C, WIN, TC), -1.0, np.float32)
    dstc[ce, lane, co_base[v_, b_] + k_] = (dloc + 128 * et)[order]
    wg = np.zeros((C, WIN, TC), np.float32)
    wg[ce, lane, go_base[v_, b_] + k_] = w[order]

    # wrap indices per gather instruction: idx i -> [i%16, off + i//16]
    TIDX = TC * WIN
    idxw = np.zeros((C, 128, TIDX // 16), np.int16)
    for (b, gc0, nch, s0) in gather_insts:
        n = nch * WIN
        segm = idxg[:, gc0 * WIN: gc0 * WIN + n].reshape(C, n // 16, 16)
        idxw[:, :16, gc0 * 8: gc0 * 8 + n // 16] = segm.transpose(0, 2, 1)
    idxw[:, 16:, :] = np.tile(idxw[:, :16, :], (1, 7, 1))

    return dict(K=K, co_base=co_base, go_base=go_base,
                gather_insts=gather_insts, sg_ranges=sg_ranges,
                bg_ranges=bg_ranges,
                TC=TC, TIDX=TIDX, NW=NW, idxw=idxw, dstc=dstc, wg=wg)


# ----------------------------------------------------------------------------
# device program
# ----------------------------------------------------------------------------

def _build_program(shapes, meta):
    DT = _dt()
    N, TW, D, OUT, NPC, NP2 = (shapes[k] for k in
                               ("N", "TW", "D", "OUT", "NPC", "NP2"))
    KT = TW // 128
    NW = meta["NW"]
    TC, TIDX = meta["TC"], meta["TIDX"]
    K, co_base, go_base = meta["K"], meta["co_base"], meta["go_base"]
    gather_insts = meta["gather_insts"]
    sg_ranges = {s0: (lo, hi) for (s0, lo, hi) in meta["sg_ranges"]}
    bg_ranges = meta["bg_ranges"]
    AF = mybir.ActivationFunctionType
    ALU = mybir.AluOpType

    nc = bacc.Bacc("TRN2", target_bir_lowering=False,
                   num_swdge_queues=N_QUEUES)

    twT = nc.dram_tensor("twT", [KT, 128, NP2], DT, kind="ExternalInput")
    idx16 = nc.dram_tensor("idx16", [128, TIDX // 16], mybir.dt.int16,
                           kind="ExternalInput")
    dstc = nc.dram_tensor("dstc", [128, TC], F32, kind="ExternalInput")
    wg = nc.dram_tensor("wg", [128, TC], F32, kind="ExternalInput")
    iot = nc.dram_tensor("iot", [128, 2 * WIN], F32, kind="ExternalInput")
    ident = nc.dram_tensor("ident", [128, 128], DT, kind="ExternalInput")
    wt = nc.dram_tensor("wt", [128, KT, 128], DT, kind="ExternalInput")
    bt = nc.dram_tensor("bt", [128, 1], F32, kind="ExternalInput")
    win = nc.dram_tensor("win", [128, 128], DT, kind="ExternalInput")
    bin_ = nc.dram_tensor("bin", [128, 1], F32, kind="ExternalInput")
    wr = nc.dram_tensor("wr", [128, R * 128], DT, kind="ExternalInput")
    root = nc.dram_tensor("root", [128, 128], DT, kind="ExternalInput")
    brg = nc.dram_tensor("brg", [128, 1], F32, kind="ExternalInput")
    w1 = nc.dram_tensor("w1", [128, 128], DT, kind="ExternalInput")
    b1 = nc.dram_tensor("b1", [128, 1], F32, kind="ExternalInput")
    w2 = nc.dram_tensor("w2", [128, OUT], DT, kind="ExternalInput")
    b2 = nc.dram_tensor("b2", [OUT, 1], F32, kind="ExternalInput")
    outT = nc.dram_tensor("outT", [OUT, NP2], F32, kind="ExternalOutput")

    with tile.TileContext(nc) as tc:
        nc.gpsimd.load_library(library_config.mlp)
        with ExitStack() as stack:
            cpool = stack.enter_context(tc.tile_pool(name="const", bufs=1))
            dpool = stack.enter_context(
                tc.tile_pool(name="dram", bufs=1, space="DRAM"))
            persist = stack.enter_context(tc.tile_pool(name="persist", bufs=1))

            def cload(dram_t, shape, dtype):
                t = cpool.tile(shape, dtype, name=f"c_{dram_t.name}")
                nc.sync.dma_start(t[:], dram_t[:])
                return t

            wt_s = cload(wt, [128, KT, 128], DT)
            bt_s = cload(bt, [128, 1], F32)
            win_s = cload(win, [128, 128], DT)
            bin_s = cload(bin_, [128, 1], F32)
            wr_s = cload(wr, [128, R * 128], DT)
            root_s = cload(root, [128, 128], DT)
            brg_s = cload(brg, [128, 1], F32)
            w1_s = cload(w1, [128, 128], DT)
            b1_s = cload(b1, [128, 1], F32)
            w2_s = cload(w2, [128, OUT], DT)
            b2_s = cload(b2, [OUT, 1], F32)
            idx_s = cload(idx16, [128, TIDX // 16], mybir.dt.int16)
            dstc_s = cload(dstc, [128, TC], F32)
            wg_s = cload(wg, [128, TC], F32)
            iot_s = cload(iot, [128, 2 * WIN], F32)
            id_s = cload(ident, [128, 128], DT)

            tables = [dpool.tile([C * NP2, 128], BF16, addr_space="Shared",
                                 name=f"table{i}") for i in range(2)]
            agins = [dpool.tile([NP2, 128], BF16, name=f"agin{i}")
                     for i in range(2)]

            xT = persist.tile([128, NP2], DT, name="xT")

            # phase A for one finished window: xT[:, win] -> node-major bf16
            # rows of agins[layer]
            def phase_a(pap, psp, layer, v):
                ps_t = psp.tile([128, 128], DT, tag="pst_a", name="ps_t")
                nc.tensor.transpose(ps_t[:], xT[:, v * 128:(v + 1) * 128],
                                    id_s[:])
                ob = pap.tile([128, 128], BF16, tag="ob", name="ob")
                nc.scalar.activation(ob[:], ps_t[:], AF.Copy)
                nc.sync.dma_start(
                    agins[layer][v * 128:(v + 1) * 128, :], ob[:])

            def fire_allgather(layer):
                nc.gpsimd.collective_compute(
                    "AllGather", mybir.AluOpType.bypass,
                    replica_groups=[list(range(C))],
                    ins=[agins[layer][:]],
                    outs=[tables[layer][:]])

            # ---------------- stage 1: x = lrelu(lrelu(tweet@Wt+bt)@Win+bin)
            # interleaved with phase A of layer 0
            with tc.tile_pool(name="s1", bufs=2) as s1p, \
                 tc.tile_pool(name="s1a", bufs=3) as s1a, \
                 tc.tile_pool(name="ps1", bufs=2, space="PSUM") as ps1, \
                 tc.tile_pool(name="ps1a", bufs=2, space="PSUM") as ps1a:
                for (c0, fw) in _col_tiles(NP2, 512):
                    twt = s1p.tile([128, KT, fw], DT, tag="twt", name="twt")
                    nc.sync.dma_start(
                        twt[:], twT[:, :, c0:c0 + fw].rearrange("k p f -> p k f"))
                    ps_t = ps1.tile([128, fw], F32, tag="pst", name="ps_t")
                    for k in range(KT):
                        nc.tensor.matmul(ps_t[:], wt_s[:, k, :], twt[:, k, :],
                                         start=(k == 0), stop=(k == KT - 1))
                    tt = s1p.tile([128, fw], DT, tag="tt", name="tt")
                    nc.scalar.activation(tt[:], ps_t[:], AF.Lrelu,
                                         bias=bt_s[:], alpha=0.01)
                    ps_x = ps1.tile([128, fw], F32, tag="psx", name="ps_x")
                    nc.tensor.matmul(ps_x[:], win_s[:], tt[:],
                                     start=True, stop=True)
                    nc.scalar.activation(xT[:, c0:c0 + fw], ps_x[:], AF.Lrelu,
                                         bias=bin_s[:], alpha=0.01)
                    for v in range(c0 // 128, (c0 + fw) // 128):
                        phase_a(s1a, ps1a, 0, v)
            fire_allgather(0)

            # ---------------- 2 RGCN layers
            for layer in range(2):
                table = tables[layer]
                per_sg = {}
                for (b, gc0, nch, s0) in gather_insts:
                    per_sg[s0] = per_sg.get(s0, 0) + 1
                g_bufs = G_BUFS or (max(per_sg.values()) + 4)
                by_sg = {}
                for gi, (b, gc0, nch, s0) in enumerate(gather_insts):
                    by_sg.setdefault(s0, []).append((b, gc0, nch, gi))
                bg_of = {}
                for (b0, lo, hi) in bg_ranges:
                    for v in range(b0, min(b0 + BG_WINDOWS, NW)):
                        bg_of[v] = (b0, lo, hi)

                with tc.tile_pool(name=f"g{layer}", bufs=g_bufs) as gp, \
                     tc.tile_pool(name=f"s{layer}", bufs=S_BUFS) as sp, \
                     tc.tile_pool(name=f"agg{layer}", bufs=3) as aggp, \
                     tc.tile_pool(name=f"pa{layer}", bufs=3) as pap, \
                     tc.tile_pool(name=f"pb{layer}", bufs=PS_BUFS,
                                  space="PSUM") as pb, \
                     tc.tile_pool(name=f"po{layer}", bufs=2,
                                  space="PSUM") as po, \
                     tc.tile_pool(name=f"pt{layer}", bufs=2,
                                  space="PSUM") as pt:
                    ssg_tiles = {}

                    def build_group(b0, lo, hi):
                        ssg = sp.tile([128, hi - lo, 2 * WIN], BF16,
                                      tag="ssg", name="ssg")
                        io3 = iot_s[:].rearrange("p (o f) -> p o f", o=1)
                        dc3 = dstc_s[:, lo:hi].rearrange(
                            "p (c o) -> p c o", o=1)
                        i_b, d_b = bass.broadcast_tensor_aps(io3, dc3)
                        nc.vector.tensor_tensor(ssg[:], i_b, d_b,
                                                ALU.is_equal)
                        ssg_tiles[b0] = ssg

                    for s0 in range(0, NW, SG_WINDOWS):
                        vs = range(s0, min(s0 + SG_WINDOWS, NW))
                        # S' builds first: no table dependency, so the DVE
                        # works through them during the AllGather
                        for v in vs:
                            b0, lo, hi = bg_of[v]
                            if b0 not in ssg_tiles:
                                build_group(b0, lo, hi)
                        gts = {}
                        for (b, gc0, nch, gi) in by_sg.get(s0, []):
                            gt = gp.tile([128, nch, 128], BF16, tag="g",
                                         name="gt")
                            nc.gpsimd.dma_gather(
                                gt[:], table[b * NP2:(b + 1) * NP2, :],
                                idx_s[:, gc0 * 8: (gc0 + nch) * 8],
                                nch * WIN, nch * WIN, 128,
                                single_packet=SINGLE_PACKET,
                                queue_num=gi % N_QUEUES)
                            # scale lanes in place by w (mean folding)
                            g3 = gt[:]
                            w3 = wg_s[:, gc0:gc0 + nch].rearrange(
                                "p (c o) -> p c o", o=1)
                            g_b, w_b = bass.broadcast_tensor_aps(g3, w3)
                            nc.vector.tensor_tensor(gt[:], g_b, w_b, ALU.mult)
                            gts.setdefault(b, []).append((gt, gc0, nch))
                        for v in vs:
                            b0, lo, hi = bg_of[v]
                            ssg = ssg_tiles[b0]
                            nv = int(K[v].sum())
                            pso = po.tile([128, WIN], F32, tag="pso",
                                          name="pso")
                            if nv:
                                ps = pb.tile([128, 2 * WIN], F32, tag="psb",
                                             name="psb")
                                i = 0
                                for b in range(C):
                                    for k in range(int(K[v, b])):
                                        ccx = int(co_base[v, b]) + k
                                        cg = int(go_base[v, b]) + k
                                        gt = None
                                        for (g_t, g_0, g_n) in gts[b]:
                                            if g_0 <= cg < g_0 + g_n:
                                                gt, j = g_t, cg - g_0
                                                break
                                        nc.tensor.matmul(
                                            ps[:], gt[:, j, :],
                                            ssg[:, ccx - lo, :],
                                            start=(i == 0),
                                            stop=(i == nv - 1))
                                        i += 1
                                # apply per-relation weights + root
                                aggS = aggp.tile([128, 2 * WIN], BF16,
                                                 tag="agg", name="aggS")
                                nc.scalar.activation(aggS[:], ps[:], AF.Copy)
                                for r in range(R):
                                    nc.tensor.matmul(
                                        pso[:],
                                        wr_s[:, r * 128:(r + 1) * 128],
                                        aggS[:, r * 128:(r + 1) * 128],
                                        start=(r == 0), stop=False)
                            nc.tensor.matmul(pso[:], root_s[:],
                                             xT[:, v * 128:(v + 1) * 128],
                                             start=(nv == 0), stop=True)
                            nc.vector.tensor_scalar(
                                xT[:, v * 128:(v + 1) * 128], pso[:],
                                brg_s[:], None, op0=ALU.add)
                            if layer == 0:
                                phase_a(pap, pt, 1, v)
                    ssg_tiles.clear()
                if layer == 0:
                    fire_allgather(1)

            # ---------------- head
            with tc.tile_pool(name="hd", bufs=3) as hp, \
                 tc.tile_pool(name="psh", bufs=2, space="PSUM") as psh, \
                 tc.tile_pool(name="outp", bufs=1) as outp:
                outT_s = outp.tile([OUT, NP2], F32, name="outT_s")
                for (c0, fw) in _col_tiles(NP2, 512):
                    ph = psh.tile([128, fw], F32, tag="ph", name="ph")
                    nc.tensor.matmul(ph[:], w1_s[:], xT[:, c0:c0 + fw],
                                     start=True, stop=True)
                    ht = hp.tile([128, fw], DT, tag="ht", name="ht")
                    nc.scalar.activation(ht[:], ph[:], AF.Lrelu,
                                         bias=b1_s[:], alpha=0.01)
                    po = psh.tile([OUT, fw], F32, tag="po", name="po")
                    nc.tensor.matmul(po[:], w2_s[:], ht[:],
                                     start=True, stop=True)
                    nc.vector.tensor_scalar(outT_s[:, c0:c0 + fw], po[:],
                                            b2_s[:], None, op0=ALU.add)
                nc.sync.dma_start(outT[:, :], outT_s[:])

    nc.compile()
    return nc


# ----------------------------------------------------------------------------
# entry point
# ----------------------------------------------------------------------------

def kernel(**inputs):
    global LAST_RESULTS
    tweet = np.asarray(inputs["tweet"], np.float32)
    ei = np.asarray(inputs["edge_index"]).astype(np.int64)
    et = np.asarray(inputs["edge_type"]).astype(np.int64)
    W_tweet = np.asarray(inputs["W_tweet"], np.float32)
    b_tweet = np.asarray(inputs["b_tweet"], np.float32)
    W_in = np.asarray(inputs["W_in"], np.float32)
    b_in = np.asarray(inputs["b_in"], np.float32)
    rgcn_weight = np.asarray(inputs["rgcn_weight"], np.float32)
    rgcn_root = np.asarray(inputs["rgcn_root"], np.float32)
    rgcn_bias = np.asarray(inputs["rgcn_bias"], np.float32)
    W_out1 = np.asarray(inputs["W_out1"], np.float32)
    b_out1 = np.asarray(inputs["b_out1"], np.float32)
    W_out2 = np.asarray(inputs["W_out2"], np.float32)
    b_out2 = np.asarray(inputs["b_out2"], np.float32)

    N, TW = tweet.shape
    D = W_in.shape[0]
    OUT = W_out2.shape[1]
    assert N % C == 0 and TW % 128 == 0 and D == 128
    NPC = N // C
    NP2 = -(-NPC // WIN) * WIN
    src, dst = ei[0], ei[1]

    meta = _edge_meta(src, dst, et, N, NPC, NP2)
    shapes = dict(N=N, TW=TW, D=D, OUT=OUT, NPC=NPC, NP2=NP2)
    npdt = _np_dt()
    KT = TW // 128

    nc = _build_program(shapes, meta)

    shared = {
        "wt": np.ascontiguousarray(
            W_tweet.reshape(KT, 128, 128).transpose(1, 0, 2)).astype(npdt),
        "bt": b_tweet.reshape(128, 1),
        "win": W_in.astype(npdt),
        "bin": b_in.reshape(128, 1),
        "wr": np.ascontiguousarray(
            rgcn_weight.transpose(1, 0, 2).reshape(128, R * 128)).astype(npdt),
        "root": rgcn_root.astype(npdt),
        "brg": rgcn_bias.reshape(128, 1),
        "w1": W_out1.astype(npdt),
        "b1": b_out1.reshape(128, 1),
        "w2": W_out2.astype(npdt),
        "b2": b_out2.reshape(OUT, 1),
        "iot": np.tile(np.arange(2 * WIN, dtype=np.float32), (128, 1)),
        "ident": np.eye(128, dtype=npdt),
    }

    in_maps = []
    for c in range(C):
        tw_c = np.zeros((KT, 128, NP2), npdt)
        tw_c[:, :, :NPC] = (tweet[c * NPC:(c + 1) * NPC].T
                            .reshape(KT, 128, NPC).astype(npdt))
        m = dict(shared)
        m["twT"] = tw_c
        m["idx16"] = meta["idxw"][c]
        m["dstc"] = meta["dstc"][c]
        m["wg"] = meta["wg"][c]
        in_maps.append(m)

    res = run_bass_kernel_spmd(nc, in_maps, core_ids=list(range(C)),
                               trace=TRACE, tmpdir=TMPDIR)
    LAST_RESULTS = res

    out = np.zeros((N, OUT), np.float32)
    for c in range(C):
        out[c * NPC:(c + 1) * NPC] = res.results[c]["outT"][:, :NPC].T
    return out


# revision 20
# speedup vs baseline: 1.4242x; 1.0726x over previous
"""Trainium2 Bass kernel for nn_BotRGCN2 (2-layer RGCN over 100k nodes / 600k edges).

Strategy (8 NeuronCores, SPMD):
  - Shard nodes across cores (12500/core, padded to 12544 = 98 windows of 128).
  - Feature-major (transposed) activations on-chip; node-major RAW-x gather
    table in DRAM (one 256B bf16 row per node -> AllGather output is
    C*NP2 x 128 = 25.7MB, half the transformed-table variant).
  - Per RGCN layer: each core transposes its xT windows (PE transpose) to
    node-major bf16 rows -> agin -> AllGather -> full raw-x table in DRAM.
    For each 128-dst-node window, dma_gather the per-edge source rows and
    scatter-add on the PE into a relation-split psum [128 feat, 256]:
      psum[:, r*128+dst] += G_w^T @ S'  with lhsT = G_w (gathered rows
    scaled in-place by w=1/cnt via a broadcast tensor_tensor) and
    rhs = S' (built ON DEVICE per 4-window group: one is_equal
    tensor_tensor of a broadcast iota [0..256) against per-lane dstcol =
    dloc + 128*rel; pad lanes have dstcol=-1 -> all-zero rows).
    Then per window: copy psum -> aggS bf16 and apply the per-relation
    weights + root with 3 accumulating matmuls:
      out = W_0^T aggS_0 + W_1^T aggS_1 + root^T xT_win (+ bias via DVE).
  - Layer-(l+1) phase A (transpose+copy+DMA of each finished window) is
    interleaved into stage 1 / layer-l phase B so the AllGather fires
    immediately after the producing phase ends.
  - Edges are preprocessed on the host: partitioned by dst owner, grouped by
    (window, src-owner-block) into 128-lane chunks, gather instructions per
    (supergroup, block) capped at 1024 indices, round-robin over 4 SWDGE
    queues.  S-structure ships as per-lane (dstcol, w) streams (tiny)
    instead of dense one-hot tables.
"""

import sys
from contextlib import ExitStack

import numpy as np

sys.path.insert(0, "/opt/trn_rl_repo")

import ml_dtypes  # noqa: E402
import concourse.bass as bass  # noqa: E402,F401
import concourse.bacc as bacc  # noqa: E402
import concourse.mybir as mybir  # noqa: E402
import concourse.tile as tile  # noqa: E402
from concourse import library_config  # noqa: E402
from concourse.bass_utils import run_bass_kernel_spmd  # noqa: E402

C = 8           # cores
WIN = 128       # dst nodes per window
R = 2           # relations

# tunables
SG_WINDOWS = 12      # windows per gather supergroup
BG_WINDOWS = 4       # windows per on-device S'-build group (divides SG_WINDOWS)
NIDX_CHUNKS_MAX = 8  # max 128-idx chunks per dma_gather (carveout limit)
SINGLE_PACKET = True
G_BUFS = 38          # gather tile buffers (2 supergroups' worth + headroom)
S_BUFS = 2           # S' build-group buffers
PS_BUFS = 4
N_QUEUES = 4         # SWDGE queues; gathers round-robin across them
USE_BF16 = True      # bf16 activations (messages are always bf16)
TRACE = False
TMPDIR = None

F32 = mybir.dt.float32
BF16 = mybir.dt.bfloat16
LAST_RESULTS = None  # BassKernelResults of the most recent run


def _dt():
    return BF16 if USE_BF16 else F32


def _np_dt():
    return ml_dtypes.bfloat16 if USE_BF16 else np.float32


def _col_tiles(total, width):
    out = []
    c = 0
    while c < total:
        out.append((c, min(width, total - c)))
        c += width
    return out


# ----------------------------------------------------------------------------
# host-side edge preprocessing
# ----------------------------------------------------------------------------

def _edge_meta(src, dst, et, N, NPC, NP2):
    """Build the SPMD-uniform chunk structure, per-core gather index streams,
    and per-core per-lane (dstcol, w) streams for on-device S' builds."""
    E = src.shape[0]
    NW = NP2 // WIN
    seg = dst * R + et
    cnt = np.bincount(seg, minlength=N * R).astype(np.float64)
    w = (1.0 / np.maximum(cnt, 1.0))[seg]

    core = dst // NPC
    nl = dst % NPC
    vwin = nl // WIN
    dloc = nl % WIN
    blk = src // NPC                       # table block == src owner core
    tloc = src % NPC                       # row within raw-x block (< NP2)
    assert NP2 <= 32768

    counts = np.zeros((C, NW, C), np.int64)
    np.add.at(counts, (core, vwin, blk), 1)
    K = -(-counts.max(axis=0) // WIN)      # [NW, C] chunks per (win, blk)

    # compute-order chunk bases (v-major, then b, then k)
    co_base = np.zeros((NW, C), np.int64)
    cc = 0
    for v in range(NW):
        for b in range(C):
            co_base[v, b] = cc
            cc += K[v, b]
    TC = int(cc)

    # gather-order (supergroup, block, window, k) + gather instruction list
    go_base = np.zeros((NW, C), np.int64)
    gather_insts = []  # (blk, start_chunk, n_chunks, sg_start)
    sg_ranges = []     # (sg_start, co_lo, co_hi)
    gc = 0
    for s0 in range(0, NW, SG_WINDOWS):
        vs = range(s0, min(s0 + SG_WINDOWS, NW))
        co_lo = int(co_base[s0, 0])
        co_hi = TC if s0 + SG_WINDOWS >= NW else int(co_base[s0 + SG_WINDOWS, 0])
        sg_ranges.append((s0, co_lo, co_hi))
        for b in range(C):
            nch = int(sum(int(K[v, b]) for v in vs))
            if nch == 0:
                continue
            off = 0
            while off < nch:
                n = min(NIDX_CHUNKS_MAX, nch - off)
                gather_insts.append((b, gc + off, n, s0))
                off += n
            for v in vs:
                go_base[v, b] = gc
                gc += K[v, b]
    assert gc == TC

    # build-group (BG_WINDOWS) compute-order chunk ranges
    bg_ranges = []     # (bg_start, co_lo, co_hi)
    for b0 in range(0, NW, BG_WINDOWS):
        lo = int(co_base[b0, 0])
        hi = TC if b0 + BG_WINDOWS >= NW else int(co_base[b0 + BG_WINDOWS, 0])
        bg_ranges.append((b0, lo, hi))

    # per-core streams
    order = np.argsort((core * NW + vwin) * C + blk, kind="stable")
    gid = ((core * NW + vwin) * C + blk)[order]
    starts = np.concatenate([[0], np.cumsum(np.bincount(gid, minlength=C * NW * C))])
    rank = np.arange(E) - starts[gid]

    ce = core[order]
    v_ = vwin[order]
    b_ = blk[order]
    k_ = rank // WIN
    lane = rank % WIN

    idxg = np.zeros((C, TC * WIN), np.int16)
    gpos = (go_base[v_, b_] + k_) * WIN + lane
    idxg[ce, gpos] = tloc[order].astype(np.int16)

    # per-lane S' structure: dstcol (compute order; -1 on pad lanes) and
    # w (gather order; 0 on pad lanes)
    dstc = np.full((C, WIN, TC), -1.0, ml_dtypes.bfloat16)
    dstc[ce, lane, co_base[v_, b_] + k_] = (dloc + 128 * et)[order].astype(
        ml_dtypes.bfloat16)
    wg = np.zeros((C, WIN, TC), ml_dtypes.bfloat16)
    wg[ce, lane, go_base[v_, b_] + k_] = w[order].astype(ml_dtypes.bfloat16)

    # wrap indices per gather instruction: idx i -> [i%16, off + i//16]
    TIDX = TC * WIN
    idxw = np.zeros((C, 128, TIDX // 16), np.int16)
    for (b, gc0, nch, s0) in gather_insts:
        n = nch * WIN
        segm = idxg[:, gc0 * WIN: gc0 * WIN + n].reshape(C, n // 16, 16)
        idxw[:, :16, gc0 * 8: gc0 * 8 + n // 16] = segm.transpose(0, 2, 1)
    idxw[:, 16:, :] = np.tile(idxw[:, :16, :], (1, 7, 1))

    return dict(K=K, co_base=co_base, go_base=go_base,
                gather_insts=gather_insts, sg_ranges=sg_ranges,
                bg_ranges=bg_ranges,
                TC=TC, TIDX=TIDX, NW=NW, idxw=idxw, dstc=dstc, wg=wg)


# ----------------------------------------------------------------------------
# device program
# ----------------------------------------------------------------------------

def _build_program(shapes, meta):
    DT = _dt()
    N, TW, D, OUT, NPC, NP2 = (shapes[k] for k in
                               ("N", "TW", "D", "OUT", "NPC", "NP2"))
    KT = TW // 128
    NW = meta["NW"]
    TC, TIDX = meta["TC"], meta["TIDX"]
    K, co_base, go_base = meta["K"], meta["co_base"], meta["go_base"]
    gather_insts = meta["gather_insts"]
    sg_ranges = {s0: (lo, hi) for (s0, lo, hi) in meta["sg_ranges"]}
    bg_ranges = meta["bg_ranges"]
    AF = mybir.ActivationFunctionType
    ALU = mybir.AluOpType

    nc = bacc.Bacc("TRN2", target_bir_lowering=False,
                   num_swdge_queues=N_QUEUES)

    twT = nc.dram_tensor("twT", [KT, 128, NP2], DT, kind="ExternalInput")
    idx16 = nc.dram_tensor("idx16", [128, TIDX // 16], mybir.dt.int16,
                           kind="ExternalInput")
    dstc = nc.dram_tensor("dstc", [128, TC], BF16, kind="ExternalInput")
    wg = nc.dram_tensor("wg", [128, TC], BF16, kind="ExternalInput")
    iot = nc.dram_tensor("iot", [128, 2 * WIN], BF16, kind="ExternalInput")
    ident = nc.dram_tensor("ident", [128, 128], DT, kind="ExternalInput")
    wt = nc.dram_tensor("wt", [128, KT, 128], DT, kind="ExternalInput")
    bt = nc.dram_tensor("bt", [128, 1], F32, kind="ExternalInput")
    win = nc.dram_tensor("win", [128, 128], DT, kind="ExternalInput")
    bin_ = nc.dram_tensor("bin", [128, 1], F32, kind="ExternalInput")
    wr = nc.dram_tensor("wr", [128, R * 128], DT, kind="ExternalInput")
    root = nc.dram_tensor("root", [128, 128], DT, kind="ExternalInput")
    brg = nc.dram_tensor("brg", [128, 1], F32, kind="ExternalInput")
    w1 = nc.dram_tensor("w1", [128, 128], DT, kind="ExternalInput")
    b1 = nc.dram_tensor("b1", [128, 1], F32, kind="ExternalInput")
    w2 = nc.dram_tensor("w2", [128, OUT], DT, kind="ExternalInput")
    b2 = nc.dram_tensor("b2", [OUT, 1], F32, kind="ExternalInput")
    outT = nc.dram_tensor("outT", [OUT, NP2], F32, kind="ExternalOutput")

    with tile.TileContext(nc) as tc:
        nc.gpsimd.load_library(library_config.mlp)
        with ExitStack() as stack:
            cpool = stack.enter_context(tc.tile_pool(name="const", bufs=1))
            dpool = stack.enter_context(
                tc.tile_pool(name="dram", bufs=1, space="DRAM"))
            persist = stack.enter_context(tc.tile_pool(name="persist", bufs=1))

            def cload(dram_t, shape, dtype):
                t = cpool.tile(shape, dtype, name=f"c_{dram_t.name}")
                nc.sync.dma_start(t[:], dram_t[:])
                return t

            wt_s = cload(wt, [128, KT, 128], DT)
            bt_s = cload(bt, [128, 1], F32)
            win_s = cload(win, [128, 128], DT)
            bin_s = cload(bin_, [128, 1], F32)
            wr_s = cload(wr, [128, R * 128], DT)
            root_s = cload(root, [128, 128], DT)
            brg_s = cload(brg, [128, 1], F32)
            w1_s = cload(w1, [128, 128], DT)
            b1_s = cload(b1, [128, 1], F32)
            w2_s = cload(w2, [128, OUT], DT)
            b2_s = cload(b2, [OUT, 1], F32)
            idx_s = cload(idx16, [128, TIDX // 16], mybir.dt.int16)
            dstc_s = cload(dstc, [128, TC], BF16)
            wg_s = cload(wg, [128, TC], BF16)
            iot_s = cload(iot, [128, 2 * WIN], BF16)
            id_s = cload(ident, [128, 128], DT)

            tables = [dpool.tile([C * NP2, 128], BF16, addr_space="Shared",
                                 name=f"table{i}") for i in range(2)]
            agins = [dpool.tile([NP2, 128], BF16, name=f"agin{i}")
                     for i in range(2)]

            xT = persist.tile([128, NP2], DT, name="xT")

            # phase A for one finished window: xT[:, win] -> node-major bf16
            # rows of agins[layer]
            def phase_a(pap, psp, layer, v):
                ps_t = psp.tile([128, 128], DT, tag="pst_a", name="ps_t")
                nc.tensor.transpose(ps_t[:], xT[:, v * 128:(v + 1) * 128],
                                    id_s[:])
                ob = pap.tile([128, 128], BF16, tag="ob", name="ob")
                nc.scalar.activation(ob[:], ps_t[:], AF.Copy)
                nc.sync.dma_start(
                    agins[layer][v * 128:(v + 1) * 128, :], ob[:])

            def fire_allgather(layer):
                nc.gpsimd.collective_compute(
                    "AllGather", mybir.AluOpType.bypass,
                    replica_groups=[list(range(C))],
                    ins=[agins[layer][:]],
                    outs=[tables[layer][:]])

            # ---------------- stage 1: x = lrelu(lrelu(tweet@Wt+bt)@Win+bin)
            # interleaved with phase A of layer 0
            with tc.tile_pool(name="s1", bufs=2) as s1p, \
                 tc.tile_pool(name="s1a", bufs=3) as s1a, \
                 tc.tile_pool(name="ps1", bufs=2, space="PSUM") as ps1, \
                 tc.tile_pool(name="ps1a", bufs=2, space="PSUM") as ps1a:
                for (c0, fw) in _col_tiles(NP2, 512):
                    twt = s1p.tile([128, KT, fw], DT, tag="twt", name="twt")
                    nc.sync.dma_start(
                        twt[:], twT[:, :, c0:c0 + fw].rearrange("k p f -> p k f"))
                    ps_t = ps1.tile([128, fw], F32, tag="pst", name="ps_t")
                    for k in range(KT):
                        nc.tensor.matmul(ps_t[:], wt_s[:, k, :], twt[:, k, :],
                                         start=(k == 0), stop=(k == KT - 1))
                    tt = s1p.tile([128, fw], DT, tag="tt", name="tt")
                    nc.scalar.activation(tt[:], ps_t[:], AF.Lrelu,
                                         bias=bt_s[:], alpha=0.01)
                    ps_x = ps1.tile([128, fw], F32, tag="psx", name="ps_x")
                    nc.tensor.matmul(ps_x[:], win_s[:], tt[:],
                                     start=True, stop=True)
                    nc.scalar.activation(xT[:, c0:c0 + fw], ps_x[:], AF.Lrelu,
                                         bias=bin_s[:], alpha=0.01)
                    for v in range(c0 // 128, (c0 + fw) // 128):
                        phase_a(s1a, ps1a, 0, v)
            fire_allgather(0)

            # ---------------- 2 RGCN layers
            for layer in range(2):
                table = tables[layer]
                g_bufs = G_BUFS
                by_sg = {}
                for gi, (b, gc0, nch, s0) in enumerate(gather_insts):
                    by_sg.setdefault(s0, []).append((b, gc0, nch, gi))
                bg_of = {}
                for (b0, lo, hi) in bg_ranges:
                    for v in range(b0, min(b0 + BG_WINDOWS, NW)):
                        bg_of[v] = (b0, lo, hi)

                with tc.tile_pool(name=f"g{layer}", bufs=g_bufs) as gp, \
                     tc.tile_pool(name=f"s{layer}", bufs=S_BUFS) as sp, \
                     tc.tile_pool(name=f"agg{layer}", bufs=3) as aggp, \
                     tc.tile_pool(name=f"pa{layer}", bufs=3) as pap, \
                     tc.tile_pool(name=f"pb{layer}", bufs=PS_BUFS,
                                  space="PSUM") as pb, \
                     tc.tile_pool(name=f"po{layer}", bufs=2,
                                  space="PSUM") as po, \
                     tc.tile_pool(name=f"pt{layer}", bufs=2,
                                  space="PSUM") as pt:
                    ssg_tiles = {}

                    def build_group(b0, lo, hi):
                        ssg = sp.tile([128, hi - lo, 2 * WIN], BF16,
                                      tag="ssg", name="ssg")
                        io3 = iot_s[:].rearrange("p (o f) -> p o f", o=1)
                        dc3 = dstc_s[:, lo:hi].rearrange(
                            "p (c o) -> p c o", o=1)
                        i_b, d_b = bass.broadcast_tensor_aps(io3, dc3)
                        nc.vector.tensor_tensor(ssg[:], i_b, d_b,
                                                ALU.is_equal)
                        ssg_tiles[b0] = ssg

                    for s0 in range(0, NW, SG_WINDOWS):
                        vs = range(s0, min(s0 + SG_WINDOWS, NW))
                        # S' builds first: no table dependency, so the DVE
                        # works through them during the AllGather
                        for v in vs:
                            b0, lo, hi = bg_of[v]
                            if b0 not in ssg_tiles:
                                build_group(b0, lo, hi)
                        gts = {}
                        for (b, gc0, nch, gi) in by_sg.get(s0, []):
                            gt = gp.tile([128, nch, 128], BF16, tag="g",
                                         name="gt")
                            nc.gpsimd.dma_gather(
                                gt[:], table[b * NP2:(b + 1) * NP2, :],
                                idx_s[:, gc0 * 8: (gc0 + nch) * 8],
                                nch * WIN, nch * WIN, 128,
                                single_packet=SINGLE_PACKET,
                                queue_num=gi % N_QUEUES)
                            # scale lanes in place by w (mean folding)
                            g3 = gt[:]
                            w3 = wg_s[:, gc0:gc0 + nch].rearrange(
                                "p (c o) -> p c o", o=1)
                            g_b, w_b = bass.broadcast_tensor_aps(g3, w3)
                            nc.vector.tensor_tensor(gt[:], g_b, w_b, ALU.mult)
                            gts.setdefault(b, []).append((gt, gc0, nch))
                        for v in vs:
                            b0, lo, hi = bg_of[v]
                            ssg = ssg_tiles[b0]
                            nv = int(K[v].sum())
                            pso = po.tile([128, WIN], F32, tag="pso",
                                          name="pso")
                            if nv:
                                ps = pb.tile([128, 2 * WIN], F32, tag="psb",
                                             name="psb")
                                i = 0
                                for b in range(C):
                                    for k in range(int(K[v, b])):
                                        ccx = int(co_base[v, b]) + k
                                        cg = int(go_base[v, b]) + k
                                        gt = None
                                        for (g_t, g_0, g_n) in gts[b]:
                                            if g_0 <= cg < g_0 + g_n:
                                                gt, j = g_t, cg - g_0
                                                break
                                        nc.tensor.matmul(
                                            ps[:], gt[:, j, :],
                                            ssg[:, ccx - lo, :],
                                            start=(i == 0),
                                            stop=(i == nv - 1))
                                        i += 1
                                # apply per-relation weights + root
                                aggS = aggp.tile([128, 2 * WIN], BF16,
                                                 tag="agg", name="aggS")
                                nc.scalar.activation(aggS[:], ps[:], AF.Copy)
                                for r in range(R):
                                    nc.tensor.matmul(
                                        pso[:],
                                        wr_s[:, r * 128:(r + 1) * 128],
                                        aggS[:, r * 128:(r + 1) * 128],
                                        start=(r == 0), stop=False)
                            nc.tensor.matmul(pso[:], root_s[:],
                                             xT[:, v * 128:(v + 1) * 128],
                                             start=(nv == 0), stop=True)
                            nc.vector.tensor_scalar(
                                xT[:, v * 128:(v + 1) * 128], pso[:],
                                brg_s[:], None, op0=ALU.add)
                            if layer == 0:
                                phase_a(pap, pt, 1, v)
                    ssg_tiles.clear()
                if layer == 0:
                    fire_allgather(1)

            # ---------------- head
            with tc.tile_pool(name="hd", bufs=3) as hp, \
                 tc.tile_pool(name="psh", bufs=2, space="PSUM") as psh, \
                 tc.tile_pool(name="outp", bufs=1) as outp:
                outT_s = outp.tile([OUT, NP2], F32, name="outT_s")
                for (c0, fw) in _col_tiles(NP2, 512):
                    ph = psh.tile([128, fw], F32, tag="ph", name="ph")
                    nc.tensor.matmul(ph[:], w1_s[:], xT[:, c0:c0 + fw],
                                     start=True, stop=True)
                    ht = hp.tile([128, fw], DT, tag="ht", name="ht")
                    nc.scalar.activation(ht[:], ph[:], AF.Lrelu,
                                         bias=b1_s[:], alpha=0.01)
                    po = psh.tile([OUT, fw], F32, tag="po", name="po")
                    nc.tensor.matmul(po[:], w2_s[:], ht[:],
                                     start=True, stop=True)
                    nc.vector.tensor_scalar(outT_s[:, c0:c0 + fw], po[:],
                                            b2_s[:], None, op0=ALU.add)
                nc.sync.dma_start(outT[:, :], outT_s[:])

    nc.compile()
    return nc


# ----------------------------------------------------------------------------
# entry point
# ----------------------------------------------------------------------------

def kernel(**inputs):
    global LAST_RESULTS
    tweet = np.asarray(inputs["tweet"], np.float32)
    ei = np.asarray(inputs["edge_index"]).astype(np.int64)
    et = np.asarray(inputs["edge_type"]).astype(np.int64)
    W_tweet = np.asarray(inputs["W_tweet"], np.float32)
    b_tweet = np.asarray(inputs["b_tweet"], np.float32)
    W_in = np.asarray(inputs["W_in"], np.float32)
    b_in = np.asarray(inputs["b_in"], np.float32)
    rgcn_weight = np.asarray(inputs["rgcn_weight"], np.float32)
    rgcn_root = np.asarray(inputs["rgcn_root"], np.float32)
    rgcn_bias = np.asarray(inputs["rgcn_bias"], np.float32)
    W_out1 = np.asarray(inputs["W_out1"], np.float32)
    b_out1 = np.asarray(inputs["b_out1"], np.float32)
    W_out2 = np.asarray(inputs["W_out2"], np.float32)
    b_out2 = np.asarray(inputs["b_out2"], np.float32)

    N, TW = tweet.shape
    D = W_in.shape[0]
    OUT = W_out2.shape[1]
    assert N % C == 0 and TW % 128 == 0 and D == 128
    NPC = N // C
    NP2 = -(-NPC // WIN) * WIN
    src, dst = ei[0], ei[1]

    meta = _edge_meta(src, dst, et, N, NPC, NP2)
    shapes = dict(N=N, TW=TW, D=D, OUT=OUT, NPC=NPC, NP2=NP2)
    npdt = _np_dt()
    KT = TW // 128

    nc = _build_program(shapes, meta)

    shared = {
        "wt": np.ascontiguousarray(
            W_tweet.reshape(KT, 128, 128).transpose(1, 0, 2)).astype(npdt),
        "bt": b_tweet.reshape(128, 1),
        "win": W_in.astype(npdt),
        "bin": b_in.reshape(128, 1),
        "wr": np.ascontiguousarray(
            rgcn_weight.transpose(1, 0, 2).reshape(128, R * 128)).astype(npdt),
        "root": rgcn_root.astype(npdt),
        "brg": rgcn_bias.reshape(128, 1),
        "w1": W_out1.astype(npdt),
        "b1": b_out1.reshape(128, 1),
        "w2": W_out2.astype(npdt),
        "b2": b_out2.reshape(OUT, 1),
        "iot": np.tile(np.arange(2 * WIN).astype(ml_dtypes.bfloat16), (128, 1)),
        "ident": np.eye(128, dtype=npdt),
    }

    in_maps = []
    for c in range(C):
        tw_c = np.zeros((KT, 128, NP2), npdt)
        tw_c[:, :, :NPC] = (tweet[c * NPC:(c + 1) * NPC].T
                            .reshape(KT, 128, NPC).astype(npdt))
        m = dict(shared)
        m["twT"] = tw_c
        m["idx16"] = meta["idxw"][c]
        m["dstc"] = meta["dstc"][c]
        m["wg"] = meta["wg"][c]
        in_maps.append(m)

    res = run_bass_kernel_spmd(nc, in_maps, core_ids=list(range(C)),
                               trace=TRACE, tmpdir=TMPDIR)
    LAST_RESULTS = res

    out = np.zeros((N, OUT), np.float32)
    for c in range(C):
        out[c * NPC:(c + 1) * NPC] = res.results[c]["outT"][:, :NPC].T
    return out
